# revision 16
# baseline (speedup 1.0000x reference)
"""Evidence-constrained self-attention on 8 TRN2 NeuronCores.

Sharding: heads across cores (2 heads/core, all 4 batches); attention is fully
local per (batch, head); context vectors are resharded with two on-chip
AllToAlls (one per local head, each overlapping remaining attention) so the
output projection is sequence-sharded (each core produces 1024 rows).

Per-core pipeline (all big operands bf16, f32 PSUM accumulation):
  1. QKV projections from host-transposed x producing Q^T/K^T [128, 8192] in
     SBUF; V PE-transposed to [k, dk] layout with an appended ones-column so
     the softmax denominator rides the PV matmul as PSUM row 64. Projection
     chunks are emitted interleaved with attention pieces (shared PSUM slots,
     retained xt tiles).
  2. Attention per (b, h) pair in two q-half passes (one 2-bank ctx tile live
     at a time): S^T = K_blk @ Q^T so softmax needs no transposes, causal
     block skipping (~45% work saved), exp on ACT with fused 1/sqrt(dk)
     scale, evidence+causal mask as a 0/1 bf16 multiply, PV accumulation in
     PSUM. Pieces are software-pipelined (QK of piece i+1 before PV of i).
  3. AllToAll of ctx^T chunks with f32 denominators bitcast into two bf16
     rows per chunk.  The output projection is split into K=64 head-halves:
     the head0 half (normalized via reciprocal + dc-selector PE broadcast)
     runs INSIDE the A2A-B window and parks bf16 partials in SBUF; after
     A2A-B lands, the head1 halves run as two concurrent PE row-group
     streams (qc0 at rows 64:128, qc1 at rows 0:63 via base-0 copies of the
     ctx and Wo halves), and a DVE add folds in the head0 partials before
     the out DMA.  A tc.no_sync_barrier() between attention and the tail
     keeps the Tile scheduler from hoisting collective-dependent tail ops
     into mid-attention queue positions (its optimistic collective estimate
     otherwise head-of-line-blocks the exp/mask pipeline for ~25us).

Workarounds for this container's toolchain: every instruction is limited to
one semaphore wait (_split_multi_waits hoists extras onto NoOps), and
collective-dependent loads use the gpsimd DMA path to avoid head-of-line
blocking the sync-engine DMA queues.
"""

import os

import numpy as np
import ml_dtypes


def _split_multi_waits(nc, max_waits: int = 1) -> int:
    """This container's walrus build allows at most ONE semaphore wait per
    instruction; Tile attaches several (notably on the kernel-tail Drain).
    Hoist all but the last wait onto single-wait NoOps inserted before the
    instruction on the same engine — semantically identical."""
    import concourse.mybir as mybir

    n_split = 0
    ctr = 0
    for f in nc.m.functions:
        stack = list(f.blocks)
        while stack:
            blk = stack.pop()
            insts = blk.instructions
            out = []
            changed = False
            for ins in insts:
                si = ins.sync_info
                if si is not None and len(si.on_wait) > max_waits:
                    waits = list(si.on_wait)
                    for w in waits[:-max_waits]:
                        nop = mybir.InstNoOp(
                            name=f"{ins.name}_wsplit{ctr}", ins=[], outs=[]
                        )
                        ctr += 1
                        nop.engine = ins.engine
                        nop.sync_info = mybir.SyncInfo(on_wait=[w], on_update=[])
                        out.append(nop)
                    si.on_wait = waits[-max_waits:]
                    changed = True
                    n_split += 1
                out.append(ins)
            if changed:
                blk.instructions = out
    return n_split

B, S, D = 4, 2048, 1024
H, DK = 16, 64
N_CORES = 8
R = B * S  # 8192 flattened rows
HPC = H // N_CORES  # heads per core = 2
DL = HPC * DK  # d_local = 128
QSH = R // N_CORES  # q rows per core after reshard = 1024
N_KB = S // 128  # 16 k-blocks per pair
N_RC = R // 512  # 16 row-chunks for projections
N_DC = D // 128  # 8 contraction chunks
# fp8 mask (1 byte/elem) is small enough to keep fully SBUF-resident
MASKV = 16.0  # fp8 mask value on masked entries; diag(-240) weights make
NEGW = -240.0  # the PE mask-add contribute 16*-240=-3840 to the logit PSUM

BF16 = ml_dtypes.bfloat16

_BUILD_CACHE = {}


def _build_nc(reps=1, loop=False):
    import concourse.bass as bass
    import concourse.mybir as mybir
    from concourse import tile
    from contextlib import ExitStack

    dt = mybir.dt
    f32 = dt.float32
    f32r = dt.float32r
    bf16 = dt.bfloat16
    fp8 = dt.float8e4
    AF = mybir.ActivationFunctionType

    nc = bass.Bass()

    xT = nc.dram_tensor("xT", [D, R], bf16, kind="ExternalInput")
    # tiny chained input so the timing harness can serialize multiple
    # executions inside one jit (defeats XLA CSE; forces ordering)
    zzdummy = nc.dram_tensor("zzdummy", [1, 4], mybir.dt.float32, kind="ExternalInput")
    zzscratch = nc.dram_tensor("zzscratch", [1, 4], mybir.dt.float32, kind="Internal")
    wqT = nc.dram_tensor("wqT", [D, DL], bf16, kind="ExternalInput")
    wkT = nc.dram_tensor("wkT", [D, DL], bf16, kind="ExternalInput")
    wvT = nc.dram_tensor("wvT", [D, DL], bf16, kind="ExternalInput")
    mask01T = nc.dram_tensor("mask01T", [S, S], fp8, kind="ExternalInput")
    negd = nc.dram_tensor("negd", [128, 128], fp8, kind="ExternalInput")
    woT = nc.dram_tensor("woT", [128, N_DC * D], bf16, kind="ExternalInput")
    sels = nc.dram_tensor("sels", [8, N_DC * 64 + N_DC * 128], bf16, kind="ExternalInput")
    ident = nc.dram_tensor("ident", [128, 128], bf16, kind="ExternalInput")
    outT = nc.dram_tensor("outT", [D, QSH], f32, kind="ExternalOutput")

    with tile.TileContext(nc) as tc, ExitStack() as ctx:
        sb = ctx.enter_context(tc.tile_pool(name="sb", bufs=1))
        psum = ctx.enter_context(tc.tile_pool(name="psum", bufs=1, space="PSUM"))
        dram = ctx.enter_context(tc.tile_pool(name="dram", bufs=1, space="DRAM"))

        # ---- persistent SBUF tensors ----
        # q^T and k^T share one tile (qt = cols 0:R, kt = cols R:2R) so the
        # projection evacuation lands both with ONE strided ACT instruction
        qkt_sb = sb.tile([128, 2 * R], bf16, name="qkt_sb")
        v_sb = sb.tile([128, N_CORES * N_KB * 65], bf16, name="v_sb")
        mask_sb = sb.tile([128, N_KB * S], fp8, name="mask_sb")
        negd_sb = sb.tile([128, 128], fp8, name="negd_sb")
        wo_sb = sb.tile([128, N_DC * D], bf16, name="wo_sb")
        a2a_sb = sb.tile([128, N_DC * QSH], bf16, name="a2a_sb")
        # head1 out-proj weights re-based at partition 0 (walrus requires
        # weight and fmap to start at the same SB partition)
        wo2_sb = sb.tile([64, N_DC * D], bf16, name="wo2_sb")
        # qc1 columns of the head1 (B-half) resharded ctx at partition base 0,
        # so the qc1 out-proj stream can run in PE row group 0 concurrently
        # with the row-group-64 qc0 stream
        a2aB2_sb = sb.tile([64, N_DC * 512], bf16, name="a2aB2_sb")
        # f32->bf16 out-proj head0 partial sums, per q-half
        part_sb = [
            sb.tile([128, N_DC * 512], bf16, name=f"part{qc}_sb") for qc in range(2)
        ]
        wq_sb = sb.tile([128, D], bf16, name="wq_sb")
        wk_sb = sb.tile([128, D], bf16, name="wk_sb")
        wv_sb = sb.tile([128, D], bf16, name="wv_sb")
        ident_sb = sb.tile([128, 128], bf16, name="ident_sb")
        # denominator-broadcast selectors: cols 0:512 = dc-selector at base 0
        # (sel2[r, dc*64+p] = r==dc), cols 512:1536 = head1 selector
        # (sel2[r, 512 + dc*128 + pp] = r==dc and pp>=64)
        sel2_sb = sb.tile([8, N_DC * 64 + N_DC * 128], bf16, name="sel2_sb")
        denA_sb = sb.tile([8, QSH], f32, name="denA_sb")
        denB_sb = sb.tile([8, QSH], f32, name="denB_sb")
        rdenA_sb = sb.tile([8, QSH], bf16, name="rdenA_sb")
        rdenB_sb = sb.tile([8, QSH], bf16, name="rdenB_sb")

        # ---- DRAM bounce buffers for the collectives (split by local head
        # so the first AllToAll overlaps attention of the second head) ----
        a2aA_in = dram.tile([N_CORES * 66, QSH], bf16, name="a2aA_in")
        a2aA_out = dram.tile([N_CORES * 66, QSH], bf16, name="a2aA_out")
        a2aB_in = dram.tile([N_CORES * 66, QSH], bf16, name="a2aB_in")
        a2aB_out = dram.tile([N_CORES * 66, QSH], bf16, name="a2aB_out")
        a2a_ins = [a2aA_in, a2aB_in]

        def emit_body(_rep):
            # ---- phase-1-critical constants: wq leads the sync queue (ahead
            # of the first xt tiles); wk/wv/ident ride the idle ACT DGE in
            # parallel so the first matmuls start ~2.5us earlier ----
            nc.sync.dma_start(
                wq_sb[:].rearrange("p (c m) -> p c m", c=N_DC),
                wqT[:].rearrange("(c p) m -> p c m", p=128),
            )
            for wsb, wdr in ((wk_sb, wkT), (wv_sb, wvT)):
                nc.scalar.dma_start(
                    wsb[:].rearrange("p (c m) -> p c m", c=N_DC),
                    wdr[:].rearrange("(c p) m -> p c m", p=128),
                )
            nc.scalar.dma_start(ident_sb[:], ident[:])
            nc.scalar.dma_start(negd_sb[:], negd[:])
            nc.sync.dma_start(zzscratch[:], zzdummy[:])
            # ones column for V_aug (data cols overwritten below)
            nc.gpsimd.memset(v_sb[:], 1.0)

            # ---- phase 1: QKV projections + V transpose (emitted in batch
            # groups, interleaved with that batch's head-0 attention) ----
            def rc_steps(rc):
                # two filler steps per projection chunk for finer interleave
                r0 = rc * 512
                st = {}

                def step_qk():
                    xts = []
                    for dc_i in range(N_DC):
                        xt = sb.tile([128, 512], bf16, name="xt", tag="xt", bufs=12)
                        nc.sync.dma_start(
                            xt[:], xT[dc_i * 128 : (dc_i + 1) * 128, r0 : r0 + 512]
                        )
                        xts.append(xt)
                    st["xts"] = xts
                    qk_ps = psum.tile([128, 1024], f32, name="qk_ps", tag="S", bufs=3)
                    for dc_i in range(N_DC):
                        first, last = dc_i == 0, dc_i == N_DC - 1
                        wslice = slice(dc_i * 128, (dc_i + 1) * 128)
                        nc.tensor.matmul(
                            qk_ps[:, 0:512], wq_sb[:, wslice], st["xts"][dc_i][:],
                            start=first, stop=last,
                        )
                        nc.tensor.matmul(
                            qk_ps[:, 512:1024], wk_sb[:, wslice], st["xts"][dc_i][:],
                            start=first, stop=last,
                        )
                    nc.scalar.activation(
                        qkt_sb[:].rearrange("p (t r) -> p t r", t=2)[:, :, r0 : r0 + 512],
                        qk_ps[:, 0:1024].rearrange("p (t c) -> p t c", t=2),
                        AF.Identity,
                    )

                def step_v():
                    v_ps = psum.tile([128, 1024], f32, name="v_ps", tag="S", bufs=3)
                    for dc_i in range(N_DC):
                        nc.tensor.matmul(
                            v_ps[:, 0:512], wv_sb[:, dc_i * 128 : (dc_i + 1) * 128],
                            st["xts"][dc_i][:], start=dc_i == 0, stop=dc_i == N_DC - 1,
                        )
                    vt_tmp = sb.tile([128, 512], bf16, name="vt_tmp", tag="vt", bufs=2)
                    nc.vector.tensor_copy(vt_tmp[:], v_ps[:, 0:512])
                    for sb4 in range(4):
                        rb = rc * 4 + sb4  # global 128-row block 0..63
                        b = rb // 16
                        kb = rb % 16
                        tr_ps = psum.tile([128, 128], bf16, name="tr_ps", tag="S", bufs=3)
                        nc.tensor.transpose(
                            tr_ps[:], vt_tmp[:, sb4 * 128 : (sb4 + 1) * 128], ident_sb[:]
                        )
                        # one strided copy covers both heads' blocks (they
                        # sit N_KB*65 apart): halves the DVE instruction count
                        # on this path and frees the tr_ps slot sooner
                        vdst = v_sb[:].rearrange(
                            "p (a k m) -> p a k m", a=2 * B, k=N_KB, m=65
                        )[:, 2 * b : 2 * b + 2, kb, 0:64]
                        nc.vector.tensor_copy(
                            vdst,
                            tr_ps[:].rearrange("p (h m) -> p h m", h=2, m=64),
                        )

                return [step_qk, step_v]

            def do_rc(rc):
                for s in rc_steps(rc):
                    s()

            # ---- phase 2: attention, head 0 pairs then head 1 pairs ----
            def do_pair(b, hl, fillers=()):
                fillers = list(fillers)
                p = b * HPC + hl
                row0 = b * S
                hs = slice(64 * hl, 64 * hl + 64)

                # two passes over q halves so only one 2-bank ctx tile is live:
                # pass 0: q in [0, 1024), kb 0..7; pass 1: q in [1024, 2048)
                # pieces: (kb, ph0, plen, evac_bank_or_None)
                passes = []
                for qlo, qhi, kbs in ((0, 1024, 8), (1024, 2048, 16)):
                    pieces = []
                    for kb in range(kbs):
                        ph0 = max(128 * kb, qlo)
                        plen = qhi - ph0
                        evac_c = None
                        if kb % 4 == 3 and qlo <= 512 * ((kb - 3) // 4) < qhi:
                            evac_c = (kb - 3) // 4
                        pieces.append((kb, ph0, plen, evac_c))
                    passes.append((qlo, qhi, pieces))

                def emit_qk(piece):
                    kb, ph0, plen, _ = piece
                    q0 = 128 * kb
                    kslice = slice(row0 + q0, row0 + q0 + 128)
                    s_ps = psum.tile([128, 1024], f32, name="s_ps", tag="S", bufs=3)
                    for sc0 in range(0, plen, 512):  # S-tile bank-aligned subs
                        slen = min(512, plen - sc0)
                        nc.tensor.matmul(
                            s_ps[:, sc0 : sc0 + slen],
                            qkt_sb[hs, R + kslice.start : R + kslice.stop],
                            qkt_sb[hs, row0 + ph0 + sc0 : row0 + ph0 + sc0 + slen],
                            start=True,
                            stop=False,
                        )
                        # evidence+causal mask rides the same PSUM group as a
                        # diag(-240) x fp8-mask matmul: masked logits get
                        # -3840 pre-exp, so exp underflows to exactly 0
                        nc.tensor.matmul(
                            s_ps[:, sc0 : sc0 + slen],
                            negd_sb[:],
                            mask_sb[:, kb * S + ph0 + sc0 : kb * S + ph0 + sc0 + slen],
                            start=False,
                            stop=True,
                        )
                    return s_ps

                def emit_rest(piece, s_ps, ctx_ps, qlo):
                    kb, ph0, plen, evac_c = piece
                    pt = sb.tile([128, 1024], bf16, name="pt", tag="pt", bufs=3)
                    nc.scalar.activation(
                        pt[:, :plen], s_ps[:, :plen], AF.Exp, scale=0.125
                    )
                    # PV accumulate; each matmul out must stay in one ctx bank
                    vbase = (p * N_KB + kb) * 65
                    g = ph0
                    while g < ph0 + plen:
                        glen = min(512 - (g % 512), ph0 + plen - g)
                        c = g // 512
                        last_kb = min(N_KB - 1, 4 * c + 3)
                        nc.tensor.matmul(
                            ctx_ps[:, g - qlo : g - qlo + glen],
                            v_sb[:, vbase : vbase + 65],
                            pt[:, g - ph0 : g - ph0 + glen],
                            start=(kb == 0),
                            stop=(kb == last_kb),
                        )
                        g += glen
                    if evac_c is not None:
                        c0 = 512 * evac_c
                        cc0 = c0 - qlo
                        ctxu = sb.tile([64, 512], bf16, name="ctxu", tag="ctxu", bufs=4)
                        nc.vector.tensor_copy(
                            ctxu[:], ctx_ps[0:64, cc0 : cc0 + 512]
                        )
                        dsb = sb.tile([65, 512], f32, name="dsb", tag="dsb", bufs=2)
                        nc.vector.tensor_copy(
                            dsb[64:65, :], ctx_ps[64:65, cc0 : cc0 + 512]
                        )
                        j = (row0 + c0) // QSH
                        t0 = (row0 + c0) % QSH
                        nc.sync.dma_start(
                            a2a_ins[hl][66 * j : 66 * j + 64, t0 : t0 + 512], ctxu[:]
                        )
                        dr = 66 * j + 64 + t0 // 512
                        nc.sync.dma_start(
                            a2a_ins[hl][dr : dr + 1, 0:1024],
                            dsb[64:65, :].bitcast(bf16),
                        )

                # interleave filler work (next batch's projection chunks)
                n_pieces = sum(len(pc) for _, _, pc in passes)
                stride = (
                    max(1, n_pieces // (len(fillers) + 1)) if fillers else 0
                )
                i = 0
                for pi, (qlo, qhi, pieces) in enumerate(passes):
                    ctx_ps = psum.tile(
                        [65, 1024], f32, name=f"ctx_{_rep}_{p}_{pi}",
                        uniquify=False, tag="ctx", bufs=1,
                    )
                    pending = []
                    for piece in pieces:
                        if fillers and i and i % stride == 0:
                            fillers.pop(0)()
                        i += 1
                        pending.append((piece, emit_qk(piece)))
                        if len(pending) > 2:
                            pc, ps = pending.pop(0)
                            emit_rest(pc, ps, ctx_ps, qlo)
                    for pc, ps in pending:
                        emit_rest(pc, ps, ctx_ps, qlo)
                for f in fillers:
                    f()

            rg = [list(range(N_CORES))]
            # batch 0 projections first, then each head-0 pair interleaved with
            # the next batch's projection chunks
            do_rc(0)
            # first resident mask blocks, after rc0's loads in queue order but
            # well before pair (0,0) consumes them
            nc.sync.dma_start(
                mask_sb[:, 0 : 4 * S].rearrange("p (c q) -> p c q", c=4),
                mask01T[0 : 4 * 128, :].rearrange("(c p) q -> p c q", p=128),
            )
            for rc in range(1, 4):
                do_rc(rc)
            nc.sync.dma_start(
                mask_sb[:, 4 * S : N_KB * S].rearrange(
                    "p (c q) -> p c q", c=N_KB - 4
                ),
                mask01T[4 * 128 : N_KB * 128, :].rearrange(
                    "(c p) q -> p c q", p=128
                ),
            )
            for b in range(B):
                rcs = range(4 * (b + 1), min(4 * (b + 2), N_RC))
                steps = [s for rc in rcs for s in rc_steps(rc)]
                do_pair(b, 0, fillers=steps)
            nc.sync.dma_start(wo_sb[:], woT[:])
            nc.sync.dma_start(wo2_sb[:], woT[64:128, :])
            nc.sync.dma_start(sel2_sb[:], sels[:])
            if not os.environ.get("K_NO_COLL"):
                nc.gpsimd.collective_compute(
                    "AllToAll", mybir.AluOpType.bypass, replica_groups=rg,
                    ins=[a2aA_in.opt()], outs=[a2aA_out.opt()],
                )
            for b in range(B):
                do_pair(b, 1)
            # scheduler-only fence: keeps every tail op behind the attention
            # ops on each engine queue (the scheduler's optimistic collective
            # estimate otherwise hoists collective-dependent tail work into
            # mid-attention queue positions, head-of-line-blocking the
            # exp/mask pipeline for ~25us)
            tc.no_sync_barrier()
            if not os.environ.get("K_NO_COLL"):
                nc.gpsimd.collective_compute(
                    "AllToAll", mybir.AluOpType.bypass, replica_groups=rg,
                    ins=[a2aB_in.opt()], outs=[a2aB_out.opt()],
                )
            # ---- tail loads (gpsimd/SWDGE so they don't head-of-line-block
            # the sync DMA queues). A-half deps are already satisfied, so the
            # first two run inside the A2A-B window. ----
            a2aA_v = a2aA_out[:].rearrange("(c p) q -> p c q", p=66)
            a2aB_v = a2aB_out[:].rearrange("(c p) q -> p c q", p=66)
            denA_f = a2aA_out[:].bitcast(f32).rearrange("(c p) q -> c p q", p=66)
            denB_f = a2aB_out[:].bitcast(f32).rearrange("(c p) q -> c p q", p=66)
            nc.gpsimd.dma_start(
                denA_sb[:].rearrange("p (a q) -> p a q", a=2), denA_f[:, 64:66, :]
            )
            nc.gpsimd.dma_start(
                a2a_sb[0:64, :].rearrange("p (c q) -> p c q", c=N_DC),
                a2aA_v[0:64, :, :],
            )
            # head0 normalization: runs inside the A2A-B window
            with nc.allow_low_precision(reason="bf16 softmax denom recip"):
                nc.vector.reciprocal(rdenA_sb[:], denA_sb[:])
            for dc_i in range(N_DC):
                bcA_ps = psum.tile([64, 1024], f32, name="bcA", tag="S", bufs=3)
                for i in range(2):
                    nc.tensor.matmul(
                        bcA_ps[:, i * 512 : (i + 1) * 512],
                        sel2_sb[:, dc_i * 64 : (dc_i + 1) * 64],
                        rdenA_sb[:, i * 512 : (i + 1) * 512],
                        start=True,
                        stop=True,
                    )
                dslice = slice(dc_i * QSH, (dc_i + 1) * QSH)
                nc.vector.tensor_mul(
                    a2a_sb[0:64, dslice], a2a_sb[0:64, dslice], bcA_ps[:]
                )

            # ---- phase 3: head-split output projection ----
            # PSUM is fully claimed by the attention pools (S: 3x2 banks,
            # ctx: 2 banks), so the W waves borrow those slots: 4 concurrent
            # [128,1024] tiles (3 from "S", 1 from "ctx") = 8 banks, each
            # packing two 512-col accumulators.
            def w_wave_tiles(label):
                tiles = []
                for i in range(4):
                    tag = "S" if i < 3 else "ctx"
                    tiles.append(
                        psum.tile(
                            [128, 1024], f32, name=f"w_{label}_{_rep}_{i}",
                            uniquify=False, tag=tag, bufs=3 if tag == "S" else 1,
                        )
                    )
                return tiles

            # head0 halves (K=64, row group 0) run inside the A2A-B window off
            # the already-loaded A-half; results parked as bf16 partials.
            for qc in range(2):
                tiles = w_wave_tiles(f"h0_{qc}")
                for ti in range(4):  # tile-outer so evacs overlap later tiles
                    for dc_i in range(N_DC):
                        for half in range(2):
                            ec = 2 * ti + half
                            co = dc_i * D + ec * 128
                            nc.tensor.matmul(
                                tiles[ti][:, half * 512 : half * 512 + 512],
                                wo_sb[0:64, co : co + 128],
                                a2a_sb[0:64, dc_i * QSH + qc * 512 : dc_i * QSH + qc * 512 + 512],
                                start=(dc_i == 0),
                                stop=(dc_i == N_DC - 1),
                            )
                    for half in range(2):
                        ec = 2 * ti + half
                        nc.scalar.activation(
                            part_sb[qc][:, ec * 512 : (ec + 1) * 512],
                            tiles[ti][:, half * 512 : half * 512 + 512],
                            AF.Identity,
                        )
            # second scheduler fence: the B-half loads and everything after
            # them stay behind the in-window head0 work on every engine queue
            # (the scheduler otherwise hoists the collective-gated loads ahead
            # of the norm-h0/partial-evac ops and head-of-line-blocks them).
            tc.no_sync_barrier()
            # B-half (head1) ctx: qc0 columns into rows 64:128 of a2a_sb, qc1
            # columns at partition base 0 (both on the gpsimd/SWDGE path; the
            # ACT/sync DGE alternatives measured worse).
            nc.gpsimd.dma_start(
                denB_sb[:].rearrange("p (a q) -> p a q", a=2), denB_f[:, 64:66, :]
            )
            nc.gpsimd.dma_start(
                a2a_sb[64:128, :].rearrange("p (c q) -> p c q", c=N_DC)[:, :, 0:512],
                a2aB_v[0:64, :, 0:512],
            )
            nc.gpsimd.dma_start(
                a2aB2_sb[:, :].rearrange("p (c q) -> p c q", c=N_DC),
                a2aB_v[0:64, :, 512:1024],
            )
            # head1 normalization (needs the A2A-B payload): qc0 at base 64 on
            # DVE, qc1 (the base-0 copy) on the Pool engine so the two halves
            # normalize in parallel.
            with nc.allow_low_precision(reason="bf16 softmax denom recip"):
                nc.vector.reciprocal(rdenB_sb[:], denB_sb[:])
            for dc_i in range(N_DC):
                bcB_ps = psum.tile([128, 512], f32, name="bcB", tag="S", bufs=3)
                nc.tensor.matmul(
                    bcB_ps[:],
                    sel2_sb[:, 512 + dc_i * 128 : 512 + (dc_i + 1) * 128],
                    rdenB_sb[:, 0:512],
                    start=True,
                    stop=True,
                )
                q0 = slice(dc_i * QSH, dc_i * QSH + 512)
                nc.vector.tensor_mul(
                    a2a_sb[64:128, q0], a2a_sb[64:128, q0], bcB_ps[64:128, :]
                )
                bcC_ps = psum.tile([64, 512], f32, name="bcC", tag="ctx", bufs=1)
                nc.tensor.matmul(
                    bcC_ps[:],
                    sel2_sb[:, dc_i * 64 : (dc_i + 1) * 64],
                    rdenB_sb[:, 512:1024],
                    start=True,
                    stop=True,
                )
                c1 = slice(dc_i * 512, (dc_i + 1) * 512)
                nc.vector.tensor_mul(a2aB2_sb[:, c1], a2aB2_sb[:, c1], bcC_ps[:])
            # head1 halves: two concurrent K=64 streams — qc0 in row group 64,
            # qc1 in row group 0 (weights stream from wo_sb partitions 64:128
            # into array rows 0:63 via the explicit tile_position override) —
            # then a DVE add folds in the head0 partial and the result DMAs out.
            for eg in range(2):  # ec groups of 4 per wave
                ecs = list(range(4 * eg, 4 * eg + 4))
                tiles = w_wave_tiles(f"h1_{eg}")
                for j, ec in enumerate(ecs):  # tile-outer: adds overlap later tiles
                    for dc_i in range(N_DC):
                        co = dc_i * D + ec * 128
                        nc.tensor.matmul(
                            tiles[j][:, 0:512],
                            wo_sb[64:128, co : co + 128],
                            a2a_sb[64:128, dc_i * QSH : dc_i * QSH + 512],
                            start=(dc_i == 0),
                            stop=(dc_i == N_DC - 1),
                        )
                        nc.tensor.matmul(
                            tiles[j][:, 512:1024],
                            wo2_sb[0:64, co : co + 128],
                            a2aB2_sb[0:64, dc_i * 512 : (dc_i + 1) * 512],
                            start=(dc_i == 0),
                            stop=(dc_i == N_DC - 1),
                        )
                    for qc in range(2):
                        out_sb = sb.tile([128, 512], f32, name="out_sb", tag="out", bufs=3)
                        nc.vector.tensor_add(
                            out_sb[:],
                            tiles[j][:, qc * 512 : qc * 512 + 512],
                            part_sb[qc][:, ec * 512 : (ec + 1) * 512],
                        )
                        nc.sync.dma_start(
                            outT[ec * 128 : (ec + 1) * 128, qc * 512 : (qc + 1) * 512],
                            out_sb[:],
                        )

        if loop and reps > 1:
            # hardware loop: one body emission executed `reps` times; used by
            # the timing harness to amplify per-body time over tunnel noise
            with tc.For_i(0, reps, 1):
                emit_body(0)
        else:
            for _rep in range(reps):
                emit_body(_rep)

    _split_multi_waits(nc)
    return nc


def get_nc():
    if "nc" not in _BUILD_CACHE:
        _BUILD_CACHE["nc"] = _build_nc()
    return _BUILD_CACHE["nc"]


def make_in_maps(hidden_states, attention_mask, Wq, Wk, Wv, Wo):
    import concourse.mybir as mybir

    FP8 = mybir.dt.np(mybir.dt.float8e4)
    hs = np.asarray(hidden_states, dtype=np.float32)
    xT = np.ascontiguousarray(hs.reshape(R, D).T.astype(BF16))
    # fp8 mask: MASKV on masked entries, 0 where allowed (transposed [k, q])
    mask01T = np.ascontiguousarray(
        np.where(np.asarray(attention_mask) != 0.0, np.float32(MASKV), 0.0)
        .T.astype(FP8)
    )
    negdm = (NEGW * np.eye(128, dtype=np.float32)).astype(FP8)
    # woT[p, dc*D + e] = Wo[e, dc*128 + p]
    woT = np.ascontiguousarray(
        np.asarray(Wo, dtype=np.float32)
        .T.reshape(N_DC, 128, D)
        .transpose(1, 0, 2)
        .reshape(128, N_DC * D)
        .astype(BF16)
    )
    # denominator-broadcast selectors (see sel2_sb comment in _build_nc)
    selsm = np.zeros((8, N_DC * 64 + N_DC * 128), dtype=np.float32)
    for dc_i in range(N_DC):
        selsm[dc_i, dc_i * 64 : (dc_i + 1) * 64] = 1.0
        selsm[dc_i, N_DC * 64 + dc_i * 128 + 64 : N_DC * 64 + (dc_i + 1) * 128] = 1.0
    selsm = selsm.astype(BF16)
    identm = np.eye(128, dtype=BF16)
    in_maps = []
    for c in range(N_CORES):
        rows = slice(c * DL, (c + 1) * DL)
        in_maps.append(
            {
                "xT": xT,
                "wqT": np.ascontiguousarray(np.asarray(Wq, np.float32)[rows].T.astype(BF16)),
                "wkT": np.ascontiguousarray(np.asarray(Wk, np.float32)[rows].T.astype(BF16)),
                "wvT": np.ascontiguousarray(np.asarray(Wv, np.float32)[rows].T.astype(BF16)),
                "mask01T": mask01T,
                "negd": negdm,
                "woT": woT,
                "sels": selsm,
                "ident": identm,
                "zzdummy": np.zeros((1, 4), np.float32),
            }
        )
    return in_maps


def assemble_output(results):
    out = np.empty((R, D), dtype=np.float32)
    for c in range(N_CORES):
        out[c * QSH : (c + 1) * QSH] = results[c]["outT"].T
    return out.reshape(B, S, D)


def kernel(hidden_states, attention_mask, Wq, Wk, Wv, Wo):
    from concourse.bass_utils import run_bass_kernel_spmd

    nc = get_nc()
    in_maps = make_in_maps(hidden_states, attention_mask, Wq, Wk, Wv, Wo)
    res = run_bass_kernel_spmd(nc, in_maps, core_ids=list(range(N_CORES)))
    return assemble_output(res.results)



# revision 17
# speedup vs baseline: 1.3171x; 1.3171x over previous
"""Evidence-constrained self-attention on 8 TRN2 NeuronCores.

Sharding: heads across cores (2 heads/core, all 4 batches); attention is fully
local per (batch, head); context vectors are resharded with two on-chip
AllToAlls (one per local head, each overlapping remaining attention) so the
output projection is sequence-sharded (each core produces 1024 rows).

Per-core pipeline (all big operands bf16, f32 PSUM accumulation):
  1. QKV projections from host-transposed x producing Q^T/K^T [128, 8192] in
     SBUF; V PE-transposed to [k, dk] layout with an appended ones-column so
     the softmax denominator rides the PV matmul as PSUM row 64. Projection
     chunks are emitted interleaved with attention pieces (shared PSUM slots,
     retained xt tiles).
  2. Attention per (b, h) pair in two q-half passes (one 2-bank ctx tile live
     at a time): S^T = K_blk @ Q^T so softmax needs no transposes, causal
     block skipping (~45% work saved), exp on ACT with fused 1/sqrt(dk)
     scale, evidence+causal mask as a 0/1 bf16 multiply, PV accumulation in
     PSUM. Pieces are software-pipelined (QK of piece i+1 before PV of i).
  3. AllToAll of ctx^T chunks with f32 denominators bitcast into two bf16
     rows per chunk.  The output projection is split into K=64 head-halves:
     the head0 half (normalized via reciprocal + dc-selector PE broadcast)
     runs INSIDE the A2A-B window and parks bf16 partials in SBUF; after
     A2A-B lands, the head1 halves run as two concurrent PE row-group
     streams (qc0 at rows 64:128, qc1 at rows 0:63 via base-0 copies of the
     ctx and Wo halves), and a DVE add folds in the head0 partials before
     the out DMA.  A tc.no_sync_barrier() between attention and the tail
     keeps the Tile scheduler from hoisting collective-dependent tail ops
     into mid-attention queue positions (its optimistic collective estimate
     otherwise head-of-line-blocks the exp/mask pipeline for ~25us).

Workarounds for this container's toolchain: every instruction is limited to
one semaphore wait (_split_multi_waits hoists extras onto NoOps), and
collective-dependent loads use the gpsimd DMA path to avoid head-of-line
blocking the sync-engine DMA queues.
"""

import os

import numpy as np
import ml_dtypes


def _split_multi_waits(nc, max_waits: int = 1) -> int:
    """This container's walrus build allows at most ONE semaphore wait per
    instruction; Tile attaches several (notably on the kernel-tail Drain).
    Hoist all but the last wait onto single-wait NoOps inserted before the
    instruction on the same engine — semantically identical."""
    import concourse.mybir as mybir

    n_split = 0
    ctr = 0
    for f in nc.m.functions:
        stack = list(f.blocks)
        while stack:
            blk = stack.pop()
            insts = blk.instructions
            out = []
            changed = False
            for ins in insts:
                si = ins.sync_info
                if si is not None and len(si.on_wait) > max_waits:
                    waits = list(si.on_wait)
                    for w in waits[:-max_waits]:
                        nop = mybir.InstNoOp(
                            name=f"{ins.name}_wsplit{ctr}", ins=[], outs=[]
                        )
                        ctr += 1
                        nop.engine = ins.engine
                        nop.sync_info = mybir.SyncInfo(on_wait=[w], on_update=[])
                        out.append(nop)
                    si.on_wait = waits[-max_waits:]
                    changed = True
                    n_split += 1
                out.append(ins)
            if changed:
                blk.instructions = out
    return n_split

B, S, D = 4, 2048, 1024
H, DK = 16, 64
N_CORES = 8
R = B * S  # 8192 flattened rows
HPC = H // N_CORES  # heads per core = 2
DL = HPC * DK  # d_local = 128
QSH = R // N_CORES  # q rows per core after reshard = 1024
N_KB = S // 128  # 16 k-blocks per pair
N_RC = R // 512  # 16 row-chunks for projections
N_DC = D // 128  # 8 contraction chunks
RESIDENT_KB = 3  # k-blocks of the mask kept SBUF-resident

BF16 = ml_dtypes.bfloat16

_BUILD_CACHE = {}


def _build_nc(reps=1, loop=False):
    import concourse.bass as bass
    import concourse.mybir as mybir
    from concourse import tile
    from contextlib import ExitStack

    dt = mybir.dt
    f32 = dt.float32
    f32r = dt.float32r
    bf16 = dt.bfloat16
    AF = mybir.ActivationFunctionType

    nc = bass.Bass()

    xT = nc.dram_tensor("xT", [D, R], bf16, kind="ExternalInput")
    # tiny chained input so the timing harness can serialize multiple
    # executions inside one jit (defeats XLA CSE; forces ordering)
    zzdummy = nc.dram_tensor("zzdummy", [1, 4], mybir.dt.float32, kind="ExternalInput")
    zzscratch = nc.dram_tensor("zzscratch", [1, 4], mybir.dt.float32, kind="Internal")
    wqT = nc.dram_tensor("wqT", [D, DL], bf16, kind="ExternalInput")
    wkT = nc.dram_tensor("wkT", [D, DL], bf16, kind="ExternalInput")
    wvT = nc.dram_tensor("wvT", [D, DL], bf16, kind="ExternalInput")
    mask01T = nc.dram_tensor("mask01T", [S, S], bf16, kind="ExternalInput")
    woT = nc.dram_tensor("woT", [128, N_DC * D], bf16, kind="ExternalInput")
    sels = nc.dram_tensor("sels", [8, N_DC * 64 + N_DC * 128], bf16, kind="ExternalInput")
    ident = nc.dram_tensor("ident", [128, 128], bf16, kind="ExternalInput")
    outT = nc.dram_tensor("outT", [D, QSH], f32, kind="ExternalOutput")

    with tile.TileContext(nc) as tc, ExitStack() as ctx:
        sb = ctx.enter_context(tc.tile_pool(name="sb", bufs=1))
        psum = ctx.enter_context(tc.tile_pool(name="psum", bufs=1, space="PSUM"))
        dram = ctx.enter_context(tc.tile_pool(name="dram", bufs=1, space="DRAM"))

        # ---- persistent SBUF tensors ----
        # q^T and k^T share one tile (qt = cols 0:R, kt = cols R:2R) so the
        # projection evacuation lands both with ONE strided ACT instruction
        qkt_sb = sb.tile([128, 2 * R], bf16, name="qkt_sb")
        v_sb = sb.tile([128, N_CORES * N_KB * 65], bf16, name="v_sb")
        mask_sb = sb.tile([128, RESIDENT_KB * S], bf16, name="mask_sb")
        wo_sb = sb.tile([128, N_DC * D], bf16, name="wo_sb")
        a2a_sb = sb.tile([128, N_DC * QSH], bf16, name="a2a_sb")
        # head1 out-proj weights re-based at partition 0 (walrus requires
        # weight and fmap to start at the same SB partition)
        wo2_sb = sb.tile([64, N_DC * D], bf16, name="wo2_sb")
        # qc1 columns of the head1 (B-half) resharded ctx at partition base 0,
        # so the qc1 out-proj stream can run in PE row group 0 concurrently
        # with the row-group-64 qc0 stream
        a2aB2_sb = sb.tile([64, N_DC * 512], bf16, name="a2aB2_sb")
        # f32->bf16 out-proj head0 partial sums, per q-half
        part_sb = [
            sb.tile([128, N_DC * 512], bf16, name=f"part{qc}_sb") for qc in range(2)
        ]
        wq_sb = sb.tile([128, D], bf16, name="wq_sb")
        wk_sb = sb.tile([128, D], bf16, name="wk_sb")
        wv_sb = sb.tile([128, D], bf16, name="wv_sb")
        ident_sb = sb.tile([128, 128], bf16, name="ident_sb")
        # denominator-broadcast selectors: cols 0:512 = dc-selector at base 0
        # (sel2[r, dc*64+p] = r==dc), cols 512:1536 = head1 selector
        # (sel2[r, 512 + dc*128 + pp] = r==dc and pp>=64)
        sel2_sb = sb.tile([8, N_DC * 64 + N_DC * 128], bf16, name="sel2_sb")
        denA_sb = sb.tile([8, QSH], f32, name="denA_sb")
        denB_sb = sb.tile([8, QSH], f32, name="denB_sb")
        rdenA_sb = sb.tile([8, QSH], bf16, name="rdenA_sb")
        rdenB_sb = sb.tile([8, QSH], bf16, name="rdenB_sb")

        # ---- DRAM bounce buffers for the collectives (split by local head
        # so the first AllToAll overlaps attention of the second head) ----
        a2aA_in = dram.tile([N_CORES * 66, QSH], bf16, name="a2aA_in")
        a2aA_out = dram.tile([N_CORES * 66, QSH], bf16, name="a2aA_out")
        a2aB_in = dram.tile([N_CORES * 66, QSH], bf16, name="a2aB_in")
        a2aB_out = dram.tile([N_CORES * 66, QSH], bf16, name="a2aB_out")
        a2a_ins = [a2aA_in, a2aB_in]

        def emit_body(_rep):
            # ---- phase-1-critical constants: wq leads the sync queue (ahead
            # of the first xt tiles); wk/wv/ident ride the idle ACT DGE in
            # parallel so the first matmuls start ~2.5us earlier ----
            nc.sync.dma_start(
                wq_sb[:].rearrange("p (c m) -> p c m", c=N_DC),
                wqT[:].rearrange("(c p) m -> p c m", p=128),
            )
            for wsb, wdr in ((wk_sb, wkT), (wv_sb, wvT)):
                nc.scalar.dma_start(
                    wsb[:].rearrange("p (c m) -> p c m", c=N_DC),
                    wdr[:].rearrange("(c p) m -> p c m", p=128),
                )
            nc.scalar.dma_start(ident_sb[:], ident[:])
            nc.sync.dma_start(zzscratch[:], zzdummy[:])
            # ones column for V_aug (data cols overwritten below)
            nc.gpsimd.memset(v_sb[:], 1.0)

            # ---- phase 1: QKV projections + V transpose (emitted in batch
            # groups, interleaved with that batch's head-0 attention) ----
            def rc_steps(rc):
                # two filler steps per projection chunk for finer interleave
                r0 = rc * 512
                st = {}

                def step_qk():
                    xts = []
                    for dc_i in range(N_DC):
                        xt = sb.tile([128, 512], bf16, name="xt", tag="xt", bufs=14)
                        nc.sync.dma_start(
                            xt[:], xT[dc_i * 128 : (dc_i + 1) * 128, r0 : r0 + 512]
                        )
                        xts.append(xt)
                    st["xts"] = xts
                    qk_ps = psum.tile([128, 1024], f32, name="qk_ps", tag="S", bufs=3)
                    for dc_i in range(N_DC):
                        first, last = dc_i == 0, dc_i == N_DC - 1
                        wslice = slice(dc_i * 128, (dc_i + 1) * 128)
                        nc.tensor.matmul(
                            qk_ps[:, 0:512], wq_sb[:, wslice], st["xts"][dc_i][:],
                            start=first, stop=last,
                        )
                        nc.tensor.matmul(
                            qk_ps[:, 512:1024], wk_sb[:, wslice], st["xts"][dc_i][:],
                            start=first, stop=last,
                        )
                    nc.scalar.activation(
                        qkt_sb[:].rearrange("p (t r) -> p t r", t=2)[:, :, r0 : r0 + 512],
                        qk_ps[:, 0:1024].rearrange("p (t c) -> p t c", t=2),
                        AF.Identity,
                    )

                def step_v():
                    v_ps = psum.tile([128, 1024], f32, name="v_ps", tag="S", bufs=3)
                    for dc_i in range(N_DC):
                        nc.tensor.matmul(
                            v_ps[:, 0:512], wv_sb[:, dc_i * 128 : (dc_i + 1) * 128],
                            st["xts"][dc_i][:], start=dc_i == 0, stop=dc_i == N_DC - 1,
                        )
                    vt_tmp = sb.tile([128, 512], bf16, name="vt_tmp", tag="vt", bufs=2)
                    nc.vector.tensor_copy(vt_tmp[:], v_ps[:, 0:512])
                    for sb4 in range(4):
                        rb = rc * 4 + sb4  # global 128-row block 0..63
                        b = rb // 16
                        kb = rb % 16
                        tr_ps = psum.tile([128, 128], bf16, name="tr_ps", tag="S", bufs=3)
                        nc.tensor.transpose(
                            tr_ps[:], vt_tmp[:, sb4 * 128 : (sb4 + 1) * 128], ident_sb[:]
                        )
                        # one strided copy covers both heads' blocks (they
                        # sit N_KB*65 apart): halves the DVE instruction count
                        # on this path and frees the tr_ps slot sooner
                        vdst = v_sb[:].rearrange(
                            "p (a k m) -> p a k m", a=2 * B, k=N_KB, m=65
                        )[:, 2 * b : 2 * b + 2, kb, 0:64]
                        nc.vector.tensor_copy(
                            vdst,
                            tr_ps[:].rearrange("p (h m) -> p h m", h=2, m=64),
                        )

                return [step_qk, step_v]

            def do_rc(rc):
                for s in rc_steps(rc):
                    s()

            # ---- phase 2: attention, head 0 pairs then head 1 pairs ----
            def do_pair(b, hl, fillers=()):
                fillers = list(fillers)
                p = b * HPC + hl
                row0 = b * S
                hs = slice(64 * hl, 64 * hl + 64)

                # two passes over q halves so only one 2-bank ctx tile is live:
                # pass 0: q in [0, 1024), kb 0..7; pass 1: q in [1024, 2048)
                # pieces: (kb, ph0, plen, evac_bank_or_None)
                passes = []
                for qlo, qhi, kbs in ((0, 1024, 8), (1024, 2048, 16)):
                    pieces = []
                    for kb in range(kbs):
                        ph0 = max(128 * kb, qlo)
                        plen = qhi - ph0
                        evac_c = None
                        if kb % 4 == 3 and qlo <= 512 * ((kb - 3) // 4) < qhi:
                            evac_c = (kb - 3) // 4
                        pieces.append((kb, ph0, plen, evac_c))
                    passes.append((qlo, qhi, pieces))

                def emit_qk(piece):
                    kb, ph0, plen, _ = piece
                    q0 = 128 * kb
                    kslice = slice(row0 + q0, row0 + q0 + 128)
                    s_ps = psum.tile([128, 1024], f32, name="s_ps", tag="S", bufs=3)
                    for sc0 in range(0, plen, 512):  # S-tile bank-aligned subs
                        slen = min(512, plen - sc0)
                        nc.tensor.matmul(
                            s_ps[:, sc0 : sc0 + slen],
                            qkt_sb[hs, R + kslice.start : R + kslice.stop],
                            qkt_sb[hs, row0 + ph0 + sc0 : row0 + ph0 + sc0 + slen],
                            start=True,
                            stop=True,
                        )
                    return s_ps

                def emit_rest(piece, s_ps, ctx_ps, qlo):
                    kb, ph0, plen, evac_c = piece
                    pt = sb.tile([128, 1024], bf16, name="pt", tag="pt", bufs=4)
                    nc.scalar.activation(
                        pt[:, :plen], s_ps[:, :plen], AF.Exp, scale=0.125
                    )
                    pm = sb.tile([128, 1024], bf16, name="pm", tag="pm", bufs=4)
                    if kb < RESIDENT_KB:
                        mtile = mask_sb[:, kb * S + ph0 : kb * S + ph0 + plen]
                    else:
                        mst = sb.tile([128, 1024], bf16, name="mst", tag="mst", bufs=4)
                        nc.sync.dma_start(
                            mst[:, :plen],
                            mask01T[kb * 128 : (kb + 1) * 128, ph0 : ph0 + plen],
                        )
                        mtile = mst[:, :plen]
                    nc.vector.tensor_mul(pm[:, :plen], pt[:, :plen], mtile)
                    # PV accumulate; each matmul out must stay in one ctx bank
                    vbase = (p * N_KB + kb) * 65
                    g = ph0
                    while g < ph0 + plen:
                        glen = min(512 - (g % 512), ph0 + plen - g)
                        c = g // 512
                        last_kb = min(N_KB - 1, 4 * c + 3)
                        nc.tensor.matmul(
                            ctx_ps[:, g - qlo : g - qlo + glen],
                            v_sb[:, vbase : vbase + 65],
                            pm[:, g - ph0 : g - ph0 + glen],
                            start=(kb == 0),
                            stop=(kb == last_kb),
                        )
                        g += glen
                    if evac_c is not None:
                        c0 = 512 * evac_c
                        cc0 = c0 - qlo
                        ctxu = sb.tile([64, 512], bf16, name="ctxu", tag="ctxu", bufs=4)
                        nc.vector.tensor_copy(
                            ctxu[:], ctx_ps[0:64, cc0 : cc0 + 512]
                        )
                        dsb = sb.tile([65, 512], f32, name="dsb", tag="dsb", bufs=2)
                        nc.vector.tensor_copy(
                            dsb[64:65, :], ctx_ps[64:65, cc0 : cc0 + 512]
                        )
                        j = (row0 + c0) // QSH
                        t0 = (row0 + c0) % QSH
                        nc.sync.dma_start(
                            a2a_ins[hl][66 * j : 66 * j + 64, t0 : t0 + 512], ctxu[:]
                        )
                        dr = 66 * j + 64 + t0 // 512
                        nc.sync.dma_start(
                            a2a_ins[hl][dr : dr + 1, 0:1024],
                            dsb[64:65, :].bitcast(bf16),
                        )

                # interleave filler work (next batch's projection chunks)
                n_pieces = sum(len(pc) for _, _, pc in passes)
                stride = (
                    max(1, n_pieces // (len(fillers) + 1)) if fillers else 0
                )
                i = 0
                for pi, (qlo, qhi, pieces) in enumerate(passes):
                    ctx_ps = psum.tile(
                        [65, 1024], f32, name=f"ctx_{_rep}_{p}_{pi}",
                        uniquify=False, tag="ctx", bufs=1,
                    )
                    pending = []
                    for piece in pieces:
                        if fillers and i and i % stride == 0:
                            fillers.pop(0)()
                        i += 1
                        pending.append((piece, emit_qk(piece)))
                        if len(pending) > 2:
                            pc, ps = pending.pop(0)
                            emit_rest(pc, ps, ctx_ps, qlo)
                    for pc, ps in pending:
                        emit_rest(pc, ps, ctx_ps, qlo)
                for f in fillers:
                    f()

            rg = [list(range(N_CORES))]
            # batch 0 projections first, then each head-0 pair interleaved with
            # the next batch's projection chunks
            do_rc(0)
            # first resident mask blocks, after rc0's loads in queue order but
            # well before pair (0,0) consumes them
            nc.sync.dma_start(
                mask_sb[:, 0 : 2 * S].rearrange("p (c q) -> p c q", c=2),
                mask01T[0 : 2 * 128, :].rearrange("(c p) q -> p c q", p=128),
            )
            for rc in range(1, 4):
                do_rc(rc)
            nc.sync.dma_start(
                mask_sb[:, 2 * S : RESIDENT_KB * S].rearrange(
                    "p (c q) -> p c q", c=RESIDENT_KB - 2
                ),
                mask01T[2 * 128 : RESIDENT_KB * 128, :].rearrange(
                    "(c p) q -> p c q", p=128
                ),
            )
            for b in range(B):
                rcs = range(4 * (b + 1), min(4 * (b + 2), N_RC))
                steps = [s for rc in rcs for s in rc_steps(rc)]
                do_pair(b, 0, fillers=steps)
            nc.sync.dma_start(wo_sb[:], woT[:])
            nc.sync.dma_start(wo2_sb[:], woT[64:128, :])
            nc.sync.dma_start(sel2_sb[:], sels[:])
            if not os.environ.get("K_NO_COLL"):
                nc.gpsimd.collective_compute(
                    "AllToAll", mybir.AluOpType.bypass, replica_groups=rg,
                    ins=[a2aA_in.opt()], outs=[a2aA_out.opt()],
                )
            for b in range(B):
                do_pair(b, 1)
            # scheduler-only fence: keeps every tail op behind the attention
            # ops on each engine queue (the scheduler's optimistic collective
            # estimate otherwise hoists collective-dependent tail work into
            # mid-attention queue positions, head-of-line-blocking the
            # exp/mask pipeline for ~25us)
            tc.no_sync_barrier()
            if not os.environ.get("K_NO_COLL"):
                nc.gpsimd.collective_compute(
                    "AllToAll", mybir.AluOpType.bypass, replica_groups=rg,
                    ins=[a2aB_in.opt()], outs=[a2aB_out.opt()],
                )
            # ---- tail loads (gpsimd/SWDGE so they don't head-of-line-block
            # the sync DMA queues). A-half deps are already satisfied, so the
            # first two run inside the A2A-B window. ----
            a2aA_v = a2aA_out[:].rearrange("(c p) q -> p c q", p=66)
            a2aB_v = a2aB_out[:].rearrange("(c p) q -> p c q", p=66)
            denA_f = a2aA_out[:].bitcast(f32).rearrange("(c p) q -> c p q", p=66)
            denB_f = a2aB_out[:].bitcast(f32).rearrange("(c p) q -> c p q", p=66)
            nc.gpsimd.dma_start(
                denA_sb[:].rearrange("p (a q) -> p a q", a=2), denA_f[:, 64:66, :]
            )
            nc.gpsimd.dma_start(
                a2a_sb[0:64, :].rearrange("p (c q) -> p c q", c=N_DC),
                a2aA_v[0:64, :, :],
            )
            # head0 normalization: runs inside the A2A-B window
            with nc.allow_low_precision(reason="bf16 softmax denom recip"):
                nc.vector.reciprocal(rdenA_sb[:], denA_sb[:])
            for dc_i in range(N_DC):
                bcA_ps = psum.tile([64, 1024], f32, name="bcA", tag="S", bufs=3)
                for i in range(2):
                    nc.tensor.matmul(
                        bcA_ps[:, i * 512 : (i + 1) * 512],
                        sel2_sb[:, dc_i * 64 : (dc_i + 1) * 64],
                        rdenA_sb[:, i * 512 : (i + 1) * 512],
                        start=True,
                        stop=True,
                    )
                dslice = slice(dc_i * QSH, (dc_i + 1) * QSH)
                nc.vector.tensor_mul(
                    a2a_sb[0:64, dslice], a2a_sb[0:64, dslice], bcA_ps[:]
                )

            # ---- phase 3: head-split output projection ----
            # PSUM is fully claimed by the attention pools (S: 3x2 banks,
            # ctx: 2 banks), so the W waves borrow those slots: 4 concurrent
            # [128,1024] tiles (3 from "S", 1 from "ctx") = 8 banks, each
            # packing two 512-col accumulators.
            def w_wave_tiles(label):
                tiles = []
                for i in range(4):
                    tag = "S" if i < 3 else "ctx"
                    tiles.append(
                        psum.tile(
                            [128, 1024], f32, name=f"w_{label}_{_rep}_{i}",
                            uniquify=False, tag=tag, bufs=3 if tag == "S" else 1,
                        )
                    )
                return tiles

            # head0 halves (K=64, row group 0) run inside the A2A-B window off
            # the already-loaded A-half; results parked as bf16 partials.
            for qc in range(2):
                tiles = w_wave_tiles(f"h0_{qc}")
                for ti in range(4):  # tile-outer so evacs overlap later tiles
                    for dc_i in range(N_DC):
                        for half in range(2):
                            ec = 2 * ti + half
                            co = dc_i * D + ec * 128
                            nc.tensor.matmul(
                                tiles[ti][:, half * 512 : half * 512 + 512],
                                wo_sb[0:64, co : co + 128],
                                a2a_sb[0:64, dc_i * QSH + qc * 512 : dc_i * QSH + qc * 512 + 512],
                                start=(dc_i == 0),
                                stop=(dc_i == N_DC - 1),
                            )
                    for half in range(2):
                        ec = 2 * ti + half
                        nc.scalar.activation(
                            part_sb[qc][:, ec * 512 : (ec + 1) * 512],
                            tiles[ti][:, half * 512 : half * 512 + 512],
                            AF.Identity,
                        )
            # second scheduler fence: the B-half loads and everything after
            # them stay behind the in-window head0 work on every engine queue
            # (the scheduler otherwise hoists the collective-gated loads ahead
            # of the norm-h0/partial-evac ops and head-of-line-blocks them).
            tc.no_sync_barrier()
            # B-half (head1) ctx: qc0 columns into rows 64:128 of a2a_sb, qc1
            # columns at partition base 0 (both on the gpsimd/SWDGE path; the
            # ACT/sync DGE alternatives measured worse).
            nc.gpsimd.dma_start(
                denB_sb[:].rearrange("p (a q) -> p a q", a=2), denB_f[:, 64:66, :]
            )
            nc.gpsimd.dma_start(
                a2a_sb[64:128, :].rearrange("p (c q) -> p c q", c=N_DC)[:, :, 0:512],
                a2aB_v[0:64, :, 0:512],
            )
            nc.gpsimd.dma_start(
                a2aB2_sb[:, :].rearrange("p (c q) -> p c q", c=N_DC),
                a2aB_v[0:64, :, 512:1024],
            )
            # head1 normalization (needs the A2A-B payload): qc0 at base 64 on
            # DVE, qc1 (the base-0 copy) on the Pool engine so the two halves
            # normalize in parallel.
            with nc.allow_low_precision(reason="bf16 softmax denom recip"):
                nc.vector.reciprocal(rdenB_sb[:], denB_sb[:])
            for dc_i in range(N_DC):
                bcB_ps = psum.tile([128, 512], f32, name="bcB", tag="S", bufs=3)
                nc.tensor.matmul(
                    bcB_ps[:],
                    sel2_sb[:, 512 + dc_i * 128 : 512 + (dc_i + 1) * 128],
                    rdenB_sb[:, 0:512],
                    start=True,
                    stop=True,
                )
                q0 = slice(dc_i * QSH, dc_i * QSH + 512)
                nc.vector.tensor_mul(
                    a2a_sb[64:128, q0], a2a_sb[64:128, q0], bcB_ps[64:128, :]
                )
                bcC_ps = psum.tile([64, 512], f32, name="bcC", tag="ctx", bufs=1)
                nc.tensor.matmul(
                    bcC_ps[:],
                    sel2_sb[:, dc_i * 64 : (dc_i + 1) * 64],
                    rdenB_sb[:, 512:1024],
                    start=True,
                    stop=True,
                )
                c1 = slice(dc_i * 512, (dc_i + 1) * 512)
                nc.vector.tensor_mul(a2aB2_sb[:, c1], a2aB2_sb[:, c1], bcC_ps[:])
            # head1 halves: two concurrent K=64 streams — qc0 in row group 64,
            # qc1 in row group 0 (weights stream from wo_sb partitions 64:128
            # into array rows 0:63 via the explicit tile_position override) —
            # then a DVE add folds in the head0 partial and the result DMAs out.
            for eg in range(2):  # ec groups of 4 per wave
                ecs = list(range(4 * eg, 4 * eg + 4))
                tiles = w_wave_tiles(f"h1_{eg}")
                for j, ec in enumerate(ecs):  # tile-outer: adds overlap later tiles
                    for dc_i in range(N_DC):
                        co = dc_i * D + ec * 128
                        nc.tensor.matmul(
                            tiles[j][:, 0:512],
                            wo_sb[64:128, co : co + 128],
                            a2a_sb[64:128, dc_i * QSH : dc_i * QSH + 512],
                            start=(dc_i == 0),
                            stop=(dc_i == N_DC - 1),
                        )
                        nc.tensor.matmul(
                            tiles[j][:, 512:1024],
                            wo2_sb[0:64, co : co + 128],
                            a2aB2_sb[0:64, dc_i * 512 : (dc_i + 1) * 512],
                            start=(dc_i == 0),
                            stop=(dc_i == N_DC - 1),
                        )
                    for qc in range(2):
                        out_sb = sb.tile([128, 512], f32, name="out_sb", tag="out", bufs=3)
                        nc.vector.tensor_add(
                            out_sb[:],
                            tiles[j][:, qc * 512 : qc * 512 + 512],
                            part_sb[qc][:, ec * 512 : (ec + 1) * 512],
                        )
                        nc.sync.dma_start(
                            outT[ec * 128 : (ec + 1) * 128, qc * 512 : (qc + 1) * 512],
                            out_sb[:],
                        )

        if loop and reps > 1:
            # hardware loop: one body emission executed `reps` times; used by
            # the timing harness to amplify per-body time over tunnel noise
            with tc.For_i(0, reps, 1):
                emit_body(0)
        else:
            for _rep in range(reps):
                emit_body(_rep)

    _split_multi_waits(nc)
    return nc


def get_nc():
    if "nc" not in _BUILD_CACHE:
        _BUILD_CACHE["nc"] = _build_nc()
    return _BUILD_CACHE["nc"]


def make_in_maps(hidden_states, attention_mask, Wq, Wk, Wv, Wo):
    hs = np.asarray(hidden_states, dtype=np.float32)
    xT = np.ascontiguousarray(hs.reshape(R, D).T.astype(BF16))
    mask01T = np.ascontiguousarray(
        (np.asarray(attention_mask) == 0.0).T.astype(BF16)
    )
    # woT[p, dc*D + e] = Wo[e, dc*128 + p]
    woT = np.ascontiguousarray(
        np.asarray(Wo, dtype=np.float32)
        .T.reshape(N_DC, 128, D)
        .transpose(1, 0, 2)
        .reshape(128, N_DC * D)
        .astype(BF16)
    )
    # denominator-broadcast selectors (see sel2_sb comment in _build_nc)
    selsm = np.zeros((8, N_DC * 64 + N_DC * 128), dtype=np.float32)
    for dc_i in range(N_DC):
        selsm[dc_i, dc_i * 64 : (dc_i + 1) * 64] = 1.0
        selsm[dc_i, N_DC * 64 + dc_i * 128 + 64 : N_DC * 64 + (dc_i + 1) * 128] = 1.0
    selsm = selsm.astype(BF16)
    identm = np.eye(128, dtype=BF16)
    in_maps = []
    for c in range(N_CORES):
        rows = slice(c * DL, (c + 1) * DL)
        in_maps.append(
            {
                "xT": xT,
                "wqT": np.ascontiguousarray(np.asarray(Wq, np.float32)[rows].T.astype(BF16)),
                "wkT": np.ascontiguousarray(np.asarray(Wk, np.float32)[rows].T.astype(BF16)),
                "wvT": np.ascontiguousarray(np.asarray(Wv, np.float32)[rows].T.astype(BF16)),
                "mask01T": mask01T,
                "woT": woT,
                "sels": selsm,
                "ident": identm,
                "zzdummy": np.zeros((1, 4), np.float32),
            }
        )
    return in_maps


def assemble_output(results):
    out = np.empty((R, D), dtype=np.float32)
    for c in range(N_CORES):
        out[c * QSH : (c + 1) * QSH] = results[c]["outT"].T
    return out.reshape(B, S, D)


def kernel(hidden_states, attention_mask, Wq, Wk, Wv, Wo):
    from concourse.bass_utils import run_bass_kernel_spmd

    nc = get_nc()
    in_maps = make_in_maps(hidden_states, attention_mask, Wq, Wk, Wv, Wo)
    res = run_bass_kernel_spmd(nc, in_maps, core_ids=list(range(N_CORES)))
    return assemble_output(res.results)

